# revision 1
# baseline (speedup 1.0000x reference)
"""ColorLoss Trainium2 kernel.

Computes mean(|blur((x+1)/2) - blur((y+1)/2)|) for x, y of shape
[32, 3, 512, 512] where blur is a separable 45-tap Gaussian (sigma=50)
with reflect padding.

Math: blur is linear, so blur(x') - blur(y') = blur((x - y)/2).
Reflect-pad + separable conv along one axis of length 512 is
multiplication by a banded 512x512 matrix A (A = C @ R with R the
reflect-padding operator and C the valid conv).  Per channel-image d:
    F = A @ d @ A.T,   answer = 0.5 * mean(|F|)
Both device matmul passes use rhs = A.T with the data as the stationary
(lhsT) operand:
    pass1: s = d^T A^T      (psum[n, m] = sum_k d[k, n] * AT[k, m])
    pass2: F = s^T...       (psum[m, j] = sum_n s[n, m] * AT[n, j])
Data parallel: 96 channel-images split 12-per-core across 8 cores; each
core returns the partial sum of |F|; the host does the tiny all-reduce.
"""

import numpy as np
import ml_dtypes
from contextlib import ExitStack

import concourse.bass as bass
import concourse.tile as tile
import concourse.mybir as mybir
from concourse import bacc
from concourse.bass import ds, ts
from concourse.bass_utils import run_bass_kernel_spmd

N_CORES = 8
IMGS_PER_CORE = 12
N = 512
KC = 4          # 128-row chunks per image
KS = 45
SIGMA = 50.0
PAD = (KS - 1) // 2
TOTAL_ELEMS = 96 * N * N

F32 = mybir.dt.float32
BF16 = mybir.dt.bfloat16


def _blur_matrix_T() -> np.ndarray:
    """A.T as [4, 128, 512] bfloat16 (AT[k, m] = A[m, k])."""
    m = (KS - 1) / 2.0
    t = np.arange(KS, dtype=np.float64)
    g = np.exp(-((t - m) ** 2) / (2.0 * SIGMA ** 2))
    g = g / g.sum()
    A = np.zeros((N, N), dtype=np.float64)
    for p in range(N + 2 * PAD):
        src = p - PAD
        if src < 0:
            src = -src
        if src > N - 1:
            src = 2 * (N - 1) - src
        for i in range(max(0, p - KS + 1), min(N, p + 1)):
            A[i, src] += g[p - i]
    AT = np.ascontiguousarray(A.T).astype(np.float32)
    return AT.reshape(KC, 128, N).astype(ml_dtypes.bfloat16)


def build(repeats: int = 1):
    """Build the per-core Bass program (all 8 cores run the same NEFF)."""
    nc = bacc.Bacc("TRN2", target_bir_lowering=False, debug=False,
                   enable_asserts=False, num_devices=N_CORES)
    x_ap = nc.dram_tensor("x", [IMGS_PER_CORE, KC, 128, N], F32,
                          kind="ExternalInput").ap()
    y_ap = nc.dram_tensor("y", [IMGS_PER_CORE, KC, 128, N], F32,
                          kind="ExternalInput").ap()
    at_ap = nc.dram_tensor("at", [KC, 128, N], BF16, kind="ExternalInput").ap()
    out_ap = nc.dram_tensor("out", [1, repeats], F32, kind="ExternalOutput").ap()

    with tile.TileContext(nc) as tc, ExitStack() as ctx:
        const_pool = ctx.enter_context(tc.tile_pool(name="const", bufs=1))
        io_pool = ctx.enter_context(tc.tile_pool(name="io", bufs=4))
        d_pool = ctx.enter_context(tc.tile_pool(name="d", bufs=3))
        s_pool = ctx.enter_context(tc.tile_pool(name="s", bufs=3))
        sc_pool = ctx.enter_context(tc.tile_pool(name="scratch", bufs=2))
        acc_pool = ctx.enter_context(tc.tile_pool(name="acc", bufs=2))
        ps1_pool = ctx.enter_context(tc.tile_pool(name="ps1", bufs=3, space="PSUM"))
        ps2_pool = ctx.enter_context(tc.tile_pool(name="ps2", bufs=3, space="PSUM"))
        psf_pool = ctx.enter_context(tc.tile_pool(name="psf", bufs=2, space="PSUM"))

        at_t = const_pool.tile([128, KC, N], BF16, name="at_t")
        for kc in range(KC):
            nc.sync.dma_start(at_t[:, kc, :], at_ap[kc])
        ones = const_pool.tile([128, 1], F32, name="ones")
        nc.vector.memset(ones[:], 1.0)
        out_t = const_pool.tile([1, repeats], F32, name="out_t")

        for r in range(repeats):
            acc = acc_pool.tile([128, 4 * IMGS_PER_CORE], F32, tag="acc")
            for i in range(IMGS_PER_CORE):
                xt = io_pool.tile([128, KC, N], F32, tag="xt")
                yt = io_pool.tile([128, KC, N], F32, tag="yt")
                for kc in range(KC):
                    nc.sync.dma_start(xt[:, kc, :], x_ap[i, kc])
                    nc.sync.dma_start(yt[:, kc, :], y_ap[i, kc])
                d = d_pool.tile([128, KC, N], BF16, tag="d")
                nc.vector.tensor_sub(d[:], xt[:], yt[:])

                s = s_pool.tile([128, KC, N], BF16, tag="s")
                for n4 in range(KC):
                    p1 = ps1_pool.tile([128, N], F32, tag="p1")
                    for kc in range(KC):
                        nc.tensor.matmul(p1[:], lhsT=d[:, kc, ts(n4, 128)],
                                         rhs=at_t[:, kc, :],
                                         start=(kc == 0), stop=(kc == KC - 1))
                    nc.scalar.copy(s[:, n4, :], p1[:])

                for mc in range(KC):
                    p2 = ps2_pool.tile([128, N], F32, tag="p2")
                    for n4 in range(KC):
                        nc.tensor.matmul(p2[:], lhsT=s[:, n4, ts(mc, 128)],
                                         rhs=at_t[:, n4, :],
                                         start=(n4 == 0), stop=(n4 == KC - 1))
                    col = i * KC + mc
                    if col % 2 == 0:
                        nc.vector.tensor_reduce(
                            acc[:, ds(col, 1)], p2[:],
                            axis=mybir.AxisListType.X, op=mybir.AluOpType.add,
                            apply_absolute_value=True)
                    else:
                        sc = sc_pool.tile([128, N], BF16, tag="sc")
                        nc.scalar.activation(
                            sc[:], p2[:], mybir.ActivationFunctionType.Abs,
                            accum_out=acc[:, ds(col, 1)])

            acc_r = acc_pool.tile([128, 1], F32, tag="accR")
            nc.vector.reduce_sum(acc_r[:], acc[:], axis=mybir.AxisListType.X)
            psf = psf_pool.tile([1, 1], F32, tag="psf")
            nc.tensor.matmul(psf[:], lhsT=acc_r[:], rhs=ones[:],
                             start=True, stop=True)
            nc.vector.tensor_copy(out_t[:, ds(r, 1)], psf[:])

        nc.sync.dma_start(out_ap[:], out_t[:])
    nc.compile()
    return nc


_CACHE: dict = {}


def _get(repeats: int = 1):
    if repeats not in _CACHE:
        _CACHE[repeats] = (build(repeats), _blur_matrix_T())
    return _CACHE[repeats]


def run_device(x: np.ndarray, y: np.ndarray, repeats: int = 1, **run_kwargs):
    """Shard, run on 8 cores, return (partial_sums_per_core, BassKernelResults)."""
    nc, at = _get(repeats)
    xs = np.ascontiguousarray(x.reshape(N_CORES, IMGS_PER_CORE, KC, 128, N))
    ys = np.ascontiguousarray(y.reshape(N_CORES, IMGS_PER_CORE, KC, 128, N))
    in_maps = [{"x": xs[c], "y": ys[c], "at": at} for c in range(N_CORES)]
    res = run_bass_kernel_spmd(nc, in_maps, core_ids=list(range(N_CORES)),
                               **run_kwargs)
    partials = np.array([res.results[c]["out"].mean() for c in range(N_CORES)])
    return partials, res


def kernel(x: np.ndarray, y: np.ndarray) -> np.ndarray:
    partials, _ = run_device(np.asarray(x, np.float32), np.asarray(y, np.float32))
    return np.float32(0.5 * partials.sum() / TOTAL_ELEMS)


# revision 4
# speedup vs baseline: 30124.6949x; 30124.6949x over previous
"""ColorLoss Trainium2 kernel.

Computes mean(|blur((x+1)/2) - blur((y+1)/2)|) for x, y of shape
[32, 3, 512, 512] where blur is a separable 45-tap Gaussian (sigma=50)
with reflect padding.

Math: blur is linear, so blur(x') - blur(y') = blur((x - y)/2).
Reflect-pad + separable conv along one axis of length 512 is
multiplication by a banded 512x512 matrix A (A = C @ R with R the
reflect-padding operator and C the valid conv).  Per channel-image d:
    F = A @ d @ A.T,   answer = 0.5 * mean(|F|)
Both device matmul passes use rhs = A.T with the data as the stationary
(lhsT) operand:
    pass1: s = d^T A^T      (psum[n, m] = sum_k d[k, n] * AT[k, m])
    pass2: F = s^T...       (psum[m, j] = sum_n s[n, m] * AT[n, j])
Data parallel: 96 channel-images split 12-per-core across 8 cores; each
core returns the partial sum of |F|; the host does the tiny all-reduce.
"""

import numpy as np
import ml_dtypes
from contextlib import ExitStack

import concourse.bass as bass
import concourse.tile as tile
import concourse.mybir as mybir
from concourse import bacc
from concourse.bass import ds, ts
from concourse.bass_utils import run_bass_kernel_spmd

N_CORES = 8
IMGS_PER_CORE = 12
N = 512
KC = 4          # 128-row chunks per image
KS = 45
SIGMA = 50.0
PAD = (KS - 1) // 2
TOTAL_ELEMS = 96 * N * N

F32 = mybir.dt.float32
BF16 = mybir.dt.bfloat16


def _blur_matrix_T() -> np.ndarray:
    """A.T as [4, 128, 512] bfloat16 (AT[k, m] = A[m, k])."""
    m = (KS - 1) / 2.0
    t = np.arange(KS, dtype=np.float64)
    g = np.exp(-((t - m) ** 2) / (2.0 * SIGMA ** 2))
    g = g / g.sum()
    A = np.zeros((N, N), dtype=np.float64)
    for p in range(N + 2 * PAD):
        src = p - PAD
        if src < 0:
            src = -src
        if src > N - 1:
            src = 2 * (N - 1) - src
        for i in range(max(0, p - KS + 1), min(N, p + 1)):
            A[i, src] += g[p - i]
    AT = np.ascontiguousarray(A.T).astype(np.float32)
    return AT.reshape(KC, 128, N).astype(ml_dtypes.bfloat16)


def build(repeats: int = 1, loop_n: int = 1):
    """Build the per-core Bass program (all 8 cores run the same NEFF).

    repeats: python-unrolled repetitions of the whole pipeline.
    loop_n: hardware For_i loop around each repetition (for benchmarking —
        re-runs identical work; result is unchanged since every iteration
        overwrites the same accumulators).
    """
    nc = bacc.Bacc("TRN2", target_bir_lowering=False, debug=False,
                   enable_asserts=False, num_devices=N_CORES)
    x_ap = nc.dram_tensor("x", [IMGS_PER_CORE, KC, 128, N], F32,
                          kind="ExternalInput").ap()
    y_ap = nc.dram_tensor("y", [IMGS_PER_CORE, KC, 128, N], F32,
                          kind="ExternalInput").ap()
    at_ap = nc.dram_tensor("at", [KC, 128, N], BF16, kind="ExternalInput").ap()
    out_ap = nc.dram_tensor("out", [1, repeats], F32, kind="ExternalOutput").ap()

    with tile.TileContext(nc) as tc, ExitStack() as ctx:
        const_pool = ctx.enter_context(tc.tile_pool(name="const", bufs=1))
        io_pool = ctx.enter_context(tc.tile_pool(name="io", bufs=4))
        d_pool = ctx.enter_context(tc.tile_pool(name="d", bufs=3))
        s_pool = ctx.enter_context(tc.tile_pool(name="s", bufs=3))
        sc_pool = ctx.enter_context(tc.tile_pool(name="scratch", bufs=2))
        acc_pool = ctx.enter_context(tc.tile_pool(name="acc", bufs=2))
        ps1_pool = ctx.enter_context(tc.tile_pool(name="ps1", bufs=3, space="PSUM"))
        ps2_pool = ctx.enter_context(tc.tile_pool(name="ps2", bufs=3, space="PSUM"))
        psf_pool = ctx.enter_context(tc.tile_pool(name="psf", bufs=2, space="PSUM"))

        at_t = const_pool.tile([128, KC, N], BF16, name="at_t")
        for kc in range(KC):
            nc.sync.dma_start(at_t[:, kc, :], at_ap[kc])
        ones = const_pool.tile([128, 1], F32, name="ones")
        nc.vector.memset(ones[:], 1.0)
        out_t = const_pool.tile([1, repeats], F32, name="out_t")

        for r in range(repeats):
            if loop_n > 1:
                loop_cm = tc.For_i(0, loop_n, 1,
                                   hint_engines=(mybir.EngineType.PE,
                                                 mybir.EngineType.SP))
                loop_cm.__enter__()
            acc = acc_pool.tile([128, 4 * IMGS_PER_CORE], F32, tag="acc")
            for i in range(IMGS_PER_CORE):
                xt = io_pool.tile([128, KC, N], F32, tag="xt")
                yt = io_pool.tile([128, KC, N], F32, tag="yt")
                for kc in range(KC):
                    nc.sync.dma_start(xt[:, kc, :], x_ap[i, kc])
                    nc.sync.dma_start(yt[:, kc, :], y_ap[i, kc])
                d = d_pool.tile([128, KC, N], BF16, tag="d")
                nc.vector.tensor_sub(d[:], xt[:], yt[:])

                s = s_pool.tile([128, KC, N], BF16, tag="s")
                for n4 in range(KC):
                    p1 = ps1_pool.tile([128, N], F32, tag="p1")
                    for kc in range(KC):
                        nc.tensor.matmul(p1[:], lhsT=d[:, kc, ts(n4, 128)],
                                         rhs=at_t[:, kc, :],
                                         start=(kc == 0), stop=(kc == KC - 1))
                    nc.scalar.copy(s[:, n4, :], p1[:])

                for mc in range(KC):
                    p2 = ps2_pool.tile([128, N], F32, tag="p2")
                    for n4 in range(KC):
                        nc.tensor.matmul(p2[:], lhsT=s[:, n4, ts(mc, 128)],
                                         rhs=at_t[:, n4, :],
                                         start=(n4 == 0), stop=(n4 == KC - 1))
                    col = i * KC + mc
                    if col % 2 == 0:
                        nc.vector.tensor_reduce(
                            acc[:, ds(col, 1)], p2[:],
                            axis=mybir.AxisListType.X, op=mybir.AluOpType.add,
                            apply_absolute_value=True)
                    else:
                        sc = sc_pool.tile([128, N], BF16, tag="sc")
                        nc.scalar.activation(
                            sc[:], p2[:], mybir.ActivationFunctionType.Abs,
                            accum_out=acc[:, ds(col, 1)])

            acc_r = acc_pool.tile([128, 1], F32, tag="accR")
            nc.vector.reduce_sum(acc_r[:], acc[:], axis=mybir.AxisListType.X)
            psf = psf_pool.tile([1, 1], F32, tag="psf")
            nc.tensor.matmul(psf[:], lhsT=acc_r[:], rhs=ones[:],
                             start=True, stop=True)
            nc.vector.tensor_copy(out_t[:, ds(r, 1)], psf[:])
            if loop_n > 1:
                loop_cm.__exit__(None, None, None)

        nc.sync.dma_start(out_ap[:], out_t[:])
    nc.compile()
    return nc


_CACHE: dict = {}


def _get(repeats: int = 1, loop_n: int = 1):
    key = (repeats, loop_n)
    if key not in _CACHE:
        _CACHE[key] = (build(repeats, loop_n), _blur_matrix_T())
    return _CACHE[key]


def run_device(x: np.ndarray, y: np.ndarray, repeats: int = 1,
               loop_n: int = 1, **run_kwargs):
    """Shard, run on 8 cores, return (partial_sums_per_core, BassKernelResults)."""
    nc, at = _get(repeats, loop_n)
    xs = np.ascontiguousarray(x.reshape(N_CORES, IMGS_PER_CORE, KC, 128, N))
    ys = np.ascontiguousarray(y.reshape(N_CORES, IMGS_PER_CORE, KC, 128, N))
    in_maps = [{"x": xs[c], "y": ys[c], "at": at} for c in range(N_CORES)]
    res = run_bass_kernel_spmd(nc, in_maps, core_ids=list(range(N_CORES)),
                               **run_kwargs)
    partials = np.array([res.results[c]["out"].mean() for c in range(N_CORES)])
    return partials, res


def kernel(x: np.ndarray, y: np.ndarray) -> np.ndarray:
    partials, _ = run_device(np.asarray(x, np.float32), np.asarray(y, np.float32))
    return np.float32(0.5 * partials.sum() / TOTAL_ELEMS)


# revision 20
# speedup vs baseline: 52546.5293x; 1.7443x over previous
"""ColorLoss Trainium2 kernel.

Computes mean(|blur((x+1)/2) - blur((y+1)/2)|) for x, y of shape
[32, 3, 512, 512] where blur is a separable 45-tap Gaussian (sigma=50)
with reflect padding.

Math: blur is linear, so blur(x') - blur(y') = blur((x - y)/2).
Reflect-pad + separable conv along one axis of length 512 is
multiplication by a banded 512x512 matrix A (A = C @ R with R the
reflect-padding operator and C the valid conv).  Per channel-image d:
    F = A @ d @ A.T,   answer = 0.5 * mean(|F|)
Both device matmul passes use rhs = A.T with the data as the stationary
(lhsT) operand:
    pass1: s = d^T A^T      (psum[n, m] = sum_k d[k, n] * AT[k, m])
    pass2: F = s^T...       (psum[m, j] = sum_n s[n, m] * AT[n, j])
Data parallel: 96 channel-images split 12-per-core across 8 cores; each
core returns the partial sum of |F|; the host does the tiny all-reduce.
"""

import numpy as np
import ml_dtypes
from contextlib import ExitStack

import concourse.bass as bass
import concourse.tile as tile
import concourse.mybir as mybir
from concourse import bacc
from concourse.bass import ds, ts
from concourse.bass_utils import run_bass_kernel_spmd

N_CORES = 8
IMGS_PER_CORE = 12
N = 512
KC = 4          # 128-row chunks per image
KS = 45
SIGMA = 50.0
PAD = (KS - 1) // 2
TOTAL_ELEMS = 96 * N * N

# Nonzero column range of each 128-row block of A^T (banded: 45-tap blur
# reaches at most +-44 columns incl. reflection).
WINDOWS = [(0, 150), (106, 278), (234, 406), (362, 512)]
# Per-block matmul segments (lo, hi, start): the psum columns each block is
# the FIRST writer of get start=True; overlap columns accumulate.  Every psum
# element is start-written exactly once, so no full-width matmul is needed.
SEGMENTS = [
    [(0, 150, True)],
    [(106, 150, False), (150, 278, True)],
    [(234, 278, False), (278, 406, True)],
    [(362, 406, False), (406, 512, True)],
]

F32 = mybir.dt.float32
BF16 = mybir.dt.bfloat16


def _blur_matrix_T() -> np.ndarray:
    """A.T as [4, 128, 512] bfloat16 (AT[k, m] = A[m, k])."""
    m = (KS - 1) / 2.0
    t = np.arange(KS, dtype=np.float64)
    g = np.exp(-((t - m) ** 2) / (2.0 * SIGMA ** 2))
    g = g / g.sum()
    A = np.zeros((N, N), dtype=np.float64)
    for p in range(N + 2 * PAD):
        src = p - PAD
        if src < 0:
            src = -src
        if src > N - 1:
            src = 2 * (N - 1) - src
        for i in range(max(0, p - KS + 1), min(N, p + 1)):
            A[i, src] += g[p - i]
    AT = np.ascontiguousarray(A.T).astype(np.float32)
    return AT.reshape(KC, 128, N).astype(ml_dtypes.bfloat16)


def build(repeats: int = 1, loop_n: int = 1):
    """Build the per-core Bass program (all 8 cores run the same NEFF).

    repeats: python-unrolled repetitions of the whole pipeline.
    loop_n: hardware For_i loop around each repetition (for benchmarking —
        re-runs identical work; result is unchanged since every iteration
        overwrites the same accumulators).
    """
    nc = bacc.Bacc("TRN2", target_bir_lowering=False, debug=False,
                   enable_asserts=False, num_devices=N_CORES)
    x_ap = nc.dram_tensor("x", [IMGS_PER_CORE, KC, 128, N], BF16,
                          kind="ExternalInput").ap()
    y_ap = nc.dram_tensor("y", [IMGS_PER_CORE, KC, 128, N], BF16,
                          kind="ExternalInput").ap()
    at_ap = nc.dram_tensor("at", [KC, 128, N], BF16, kind="ExternalInput").ap()
    out_ap = nc.dram_tensor("out", [1, repeats], F32, kind="ExternalOutput").ap()

    with tile.TileContext(nc) as tc, ExitStack() as ctx:
        const_pool = ctx.enter_context(tc.tile_pool(name="const", bufs=1))
        io_pool = ctx.enter_context(tc.tile_pool(name="io", bufs=6))
        d_pool = ctx.enter_context(tc.tile_pool(name="d", bufs=3))
        s_pool = ctx.enter_context(tc.tile_pool(name="s", bufs=3))
        sc_pool = ctx.enter_context(tc.tile_pool(name="scratch", bufs=2))
        acc_pool = ctx.enter_context(tc.tile_pool(name="acc", bufs=2))
        ps1_pool = ctx.enter_context(tc.tile_pool(name="ps1", bufs=3, space="PSUM"))
        ps2_pool = ctx.enter_context(tc.tile_pool(name="ps2", bufs=3, space="PSUM"))
        psf_pool = ctx.enter_context(tc.tile_pool(name="psf", bufs=2, space="PSUM"))

        at_t = const_pool.tile([128, KC, N], BF16, name="at_t")
        nc.sync.dma_start(at_t[:], at_ap[:].transpose([1, 0, 2]))
        ones = const_pool.tile([128, 1], F32, name="ones")
        nc.vector.memset(ones[:], 1.0)
        out_t = const_pool.tile([1, repeats], F32, name="out_t")

        for r in range(repeats):
            if loop_n > 1:
                loop_cm = tc.For_i(0, loop_n, 1,
                                   hint_engines=(mybir.EngineType.PE,
                                                 mybir.EngineType.SP))
                loop_cm.__enter__()
            acc = acc_pool.tile([128, 4 * IMGS_PER_CORE], F32, tag="acc")
            for i in range(IMGS_PER_CORE):
                xt = io_pool.tile([128, KC, N], BF16, tag="xt")
                yt = io_pool.tile([128, KC, N], BF16, tag="yt")
                # half-image DMAs: finer arrival granularity shortens the
                # end-of-stream compute tail
                for h in range(2):
                    hs = ts(h, 2)
                    nc.sync.dma_start(xt[:, hs, :], x_ap[i, hs].transpose([1, 0, 2]))
                    nc.sync.dma_start(yt[:, hs, :], y_ap[i, hs].transpose([1, 0, 2]))
                d = d_pool.tile([128, KC, N], BF16, tag="d")
                # per-chunk subtracts split DVE/GpSimd: balances engines and
                # lets pass1 start as soon as chunk 0 is ready
                for kc in range(KC):
                    sub_eng = nc.vector if (kc + i) % 2 else nc.gpsimd
                    sub_eng.tensor_sub(d[:, kc, :], xt[:, kc, :], yt[:, kc, :])

                s = s_pool.tile([128, KC, N], BF16, tag="s")
                for n4 in range(KC):
                    p1 = ps1_pool.tile([128, N], F32, tag="p1")
                    for kc in range(KC):
                        for lo, hi, st in SEGMENTS[kc]:
                            nc.tensor.matmul(p1[:, lo:hi],
                                             lhsT=d[:, kc, ts(n4, 128)],
                                             rhs=at_t[:, kc, lo:hi],
                                             start=st, stop=(kc == KC - 1))
                    scopy_eng = nc.vector if n4 % 2 else nc.scalar
                    if scopy_eng is nc.vector:
                        nc.vector.tensor_copy(s[:, n4, :], p1[:])
                    else:
                        nc.scalar.copy(s[:, n4, :], p1[:])

                for mc in range(KC):
                    p2 = ps2_pool.tile([128, N], F32, tag="p2")
                    for n4 in range(KC):
                        for lo, hi, st in SEGMENTS[n4]:
                            nc.tensor.matmul(p2[:, lo:hi],
                                             lhsT=s[:, n4, ts(mc, 128)],
                                             rhs=at_t[:, n4, lo:hi],
                                             start=st, stop=(n4 == KC - 1))
                    col = i * KC + mc
                    if col % 2 == 0:
                        nc.vector.tensor_reduce(
                            acc[:, ds(col, 1)], p2[:],
                            axis=mybir.AxisListType.X, op=mybir.AluOpType.add,
                            apply_absolute_value=True)
                    else:
                        sc = sc_pool.tile([128, N], BF16, tag="sc")
                        nc.scalar.activation(
                            sc[:], p2[:], mybir.ActivationFunctionType.Abs,
                            accum_out=acc[:, ds(col, 1)])

            acc_r = acc_pool.tile([128, 1], F32, tag="accR")
            nc.vector.reduce_sum(acc_r[:], acc[:], axis=mybir.AxisListType.X)
            psf = psf_pool.tile([1, 1], F32, tag="psf")
            nc.tensor.matmul(psf[:], lhsT=acc_r[:], rhs=ones[:],
                             start=True, stop=True)
            nc.vector.tensor_copy(out_t[:, ds(r, 1)], psf[:])
            if loop_n > 1:
                loop_cm.__exit__(None, None, None)

        nc.sync.dma_start(out_ap[:], out_t[:])
    nc.compile()
    return nc


_CACHE: dict = {}


def _get(repeats: int = 1, loop_n: int = 1):
    key = (repeats, loop_n)
    if key not in _CACHE:
        _CACHE[key] = (build(repeats, loop_n), _blur_matrix_T())
    return _CACHE[key]


def run_device(x: np.ndarray, y: np.ndarray, repeats: int = 1,
               loop_n: int = 1, **run_kwargs):
    """Shard, run on 8 cores, return (partial_sums_per_core, BassKernelResults)."""
    nc, at = _get(repeats, loop_n)
    xs = x.reshape(N_CORES, IMGS_PER_CORE, KC, 128, N).astype(ml_dtypes.bfloat16)
    ys = y.reshape(N_CORES, IMGS_PER_CORE, KC, 128, N).astype(ml_dtypes.bfloat16)
    in_maps = [{"x": xs[c], "y": ys[c], "at": at} for c in range(N_CORES)]
    res = run_bass_kernel_spmd(nc, in_maps, core_ids=list(range(N_CORES)),
                               **run_kwargs)
    partials = np.array([res.results[c]["out"].mean() for c in range(N_CORES)])
    return partials, res


def kernel(x: np.ndarray, y: np.ndarray) -> np.ndarray:
    partials, _ = run_device(np.asarray(x, np.float32), np.asarray(y, np.float32))
    return np.float32(0.5 * partials.sum() / TOTAL_ELEMS)
